# revision 2
# baseline (speedup 1.0000x reference)
"""Trainium2 Bass kernel for nn_GaussianKernel (embedding_lookup / ridge) — v2.

Computation (B=16 batches, N=256 tokens, K=128 RBF centers, H=16 out):
    gamma = gamma_table[tok_i, tok_j]; beta = beta_table[tok_i, tok_j]
    s     = gamma * d + beta                                  (B,N,N)
    psi_k = exp(-((s-mu_k)^2)/(2 sigma_k^2)) / (sqrt(2pi) sigma_k)
    h     = relu(psi @ W1 + b1); phi = h @ W2 + b2            (B,N,N,H)
    out   = transpose -> (B,H,N,N)

phi is a fixed 16-vector function of the scalar s.  The host fits it as
phi(s) ~= c0 + sum_m coef[m,:] relu(u - k_m)  (u = d; 32 optimized knots;
the const c0 folds into the PSUM drain).  Device work per 2048-pair
"qslab" (4 slabs of 512 pairs, d staged as u4[b, 512q+c] = d[slab 4q+b]):
  - ONE rank-4 matmul with a [4,128] selector stationary broadcasts the 4
    slabs' u into a [128,512] PSUM tile (partition 32B+m = slab-block B,
    basis row m) — 512 PE cycles for 2048 pairs
  - ONE ACT (Relu, bias=-k per partition) or DVE (add,max) instruction
    evaluates all 32 basis rows -> f32r R in SBUF
  - ONE f32r matmul with a shifted block stationary contracts 32 basis
    rows -> 16 outputs for 4 slabs at once; 2 qslabs accumulate into one
    PSUM bank laid out [slab t -> partitions 16t..16t+16]
  - Pool drains PSUM->SBUF adding the folded c0; one DMA per 2 groups.
Engine busy (cost model, per core, 131072 pairs): PE 27us, DMA 26us,
Pool 23us, ACT 18us, DVE 19us.

The general (non-constant tables) path is the v1 kernel, kept verbatim.
"""

import numpy as np

import concourse.bass as bass
import concourse.mybir as mybir
import concourse.tile as tile
from concourse import bacc
from concourse.bass import ds
from concourse.bass_utils import run_bass_kernel_spmd

B, N, T, K, H = 16, 256, 128, 128, 16
NCORES = 8
BPC = B // NCORES          # batches per core
F32 = mybir.dt.float32
F32R = mybir.dt.float32r
F16 = mybir.dt.float16
AF = mybir.ActivationFunctionType
ALU = mybir.AluOpType

M = 32                     # basis rows per slab
NQ = 32                    # qslabs (2048 pairs) per batch
CW = 512                   # consts tile width (f32 cols)
# consts layout (f32 cols): S0 0:128 | S1 128:256 | sel4 rows0:4 256:384 |
# bias(-knots) col 384 | dcon col 385 | warm col 386+
SC_BIAS, SC_DCON, SC_WARM = 384, 385, 386


# ---------------------------------------------------------------------------
# trivial path (gamma/beta constant)
# ---------------------------------------------------------------------------
def _build_nc_trivial(cfg=None):
    """M=16-block variant: per "oslab" (8 slabs = 4096 pairs) one rank-8
    selector matmul broadcasts u into [128,512] PSUM (partition 16*t + m),
    TWO basis ops (relu with bias sets A/B) make re_A/re_B, two block-diag
    stationary matmuls contract both 16-row sets into one PSUM bank
    [slab t -> partitions 16t..16t+16], ACT/DVE drain adds the folded
    constant, one DMA per oslab writes 256KB."""
    cfg = dict(cfg or {})
    EA = cfg.get("EA", 7)          # bcast-matmul lookahead (oslabs)
    BA = cfg.get("BA", 3)          # basis lookahead
    uw = cfg.get("uw", 4)          # oslabs per u8 staging chunk
    UA = cfg.get("UA", 8)          # u8 chunk dma lookahead (oslabs)
    u4_bufs = cfg.get("u4_bufs", 3)
    act_share = cfg.get("act_share", 2)   # of every 4 basis ops, on ACT
    dr_act = cfg.get("dr_act", 2)  # of every 4 drains, on ACT (rest DVE)
    ad8 = cfg.get("ad8", None)     # optional: of every 8 drains, on ACT
    pool_dma = cfg.get("pool_dma", 0)  # of every 4 out DMAs, on Pool swdge
    rr_bufs = cfg.get("rr_bufs", 20)
    op_bufs = cfg.get("op_bufs", 12)
    out_delay = cfg.get("out_delay", 4)
    pse_bufs = cfg.get("pse_bufs", 4)
    psc_bufs = cfg.get("psc_bufs", 4)
    ue_w = cfg.get("ue_w", 1)      # oslabs per ue PSUM tile
    psc_w = cfg.get("psc_w", 1)    # oslabs per psc PSUM tile
    NO = 16                        # oslabs per batch

    nc = bacc.Bacc("TRN2", target_bir_lowering=False)

    d_in = nc.dram_tensor("d", [BPC, N, N], F32R, kind="ExternalInput")
    c_d = nc.dram_tensor("consts", [128, CW], F32R, kind="ExternalInput")
    out_d = nc.dram_tensor("out", [BPC, H, N, N], F32, kind="ExternalOutput")
    d_flat = d_in.rearrange("b i j -> b (i j)")

    with tile.TileContext(nc) as tc:
        with (
            tc.tile_pool(name="consts", bufs=1) as cpool,
            tc.tile_pool(name="u8", bufs=u4_bufs) as u8pool,
            tc.tile_pool(name="rr", bufs=rr_bufs) as rpool,
            tc.tile_pool(name="outp", bufs=op_bufs) as opool,
            tc.tile_pool(name="ps_e", bufs=pse_bufs, space="PSUM") as ps_e,
            tc.tile_pool(name="ps_c", bufs=psc_bufs, space="PSUM") as ps_c,
        ):
            C = cpool.tile([128, CW], F32R)
            nc.sync.dma_start(out=C[:, 256:400], in_=c_d[:, 256:400])
            nc.sync.dma_start(out=C[:, 0:256], in_=c_d[:, 0:256])
            biasA = C[:, 384:385].bitcast(F32)
            biasB = C[:, 385:386].bitcast(F32)
            dcon_ap = C[:, 386:387].bitcast(F32)
            s_sb = [C[:, 0:128], C[:, 128:256]]
            sel8 = C[0:8, 256:384]
            WARM = 388

            # warm-up: PE/ACT/DVE touch C once so later instructions need
            # only one sync wait (matmult can hold just one)
            wus = cpool.tile([1, 32], F32)
            nc.scalar.copy(out=wus[:, 0:8], in_=C[0:1, WARM:WARM + 8])
            nc.vector.tensor_scalar(
                out=wus[:, 8:16], in0=C[0:1, WARM:WARM + 8],
                scalar1=0.0, scalar2=None, op0=ALU.add)
            wu = ps_e.tile([1, 8], F32, tag="e")
            nc.tensor.matmul(wu, C[0:1, WARM:WARM + 1],
                             C[0:1, WARM:WARM + 8],
                             start=True, stop=True)
            nc.vector.tensor_scalar(
                out=wus[:, 24:32], in0=wu, scalar1=0.0, scalar2=None,
                op0=ALU.add)

            u8 = {}

            def emit_u8(bb, ch):
                def fn():
                    ut = u8pool.tile([8, uw * 512], F32R, name="u8t")
                    nc.sync.dma_start(
                        out=ut.rearrange("b (o c) -> b o c", o=uw),
                        in_=d_flat[bb, ds(4096 * uw * ch, 4096 * uw)]
                        .rearrange("(o b c) -> b o c", o=uw, b=8, c=512))
                    for oo in range(uw):
                        u8[(bb, uw * ch + oo)] = ut
                return fn

            from collections import defaultdict
            actions = defaultdict(list)
            PRE = 10
            interleave = cfg.get("interleave", False) and BPC == 2
            LAST = BPC * NO - 1

            def at(bb, o, prio, fn):
                if interleave:
                    step = 2 * o + bb
                else:
                    step = bb * NO + o
                actions[min(max(step, -PRE), LAST)].append((prio, fn))

            Rinfo = {}
            state = {}
            bk = [0]
            drk = [0]
            odk = [0]

            def emit_bcast(bb, o):
                def fn():
                    if o % ue_w == 0:
                        state["ue"] = ps_e.tile([128, 512 * ue_w], F32,
                                                tag="e", name="ue")
                    ue = state["ue"]
                    ut = u8.pop((bb, o))
                    nc.tensor.matmul(ue[:, ds(512 * (o % ue_w), 512)], sel8,
                                     ut[:, ds(512 * (o % uw), 512)],
                                     start=True, stop=True)
                    Rinfo[(bb, o, "u")] = ue
                return fn

            def emit_basis(bb, o):
                def fn():
                    if o % ue_w != ue_w - 1:
                        return
                    o0 = o - (ue_w - 1)
                    ue = Rinfo.pop((bb, o0, "u"))
                    for oo in range(o0 + 1, o + 1):
                        Rinfo.pop((bb, oo, "u"), None)
                    for x, bias in (("A", biasA), ("B", biasB)):
                        re = rpool.tile([128, 512 * ue_w], F32R, name="re")
                        on_act = (x == "A") if act_share == 2 else (
                            bk[0] % 4 < act_share)
                        if on_act:
                            nc.scalar.activation(out=re, in_=ue,
                                                 func=AF.Relu, bias=bias)
                        else:
                            nc.vector.tensor_scalar(
                                out=re, in0=ue, scalar1=bias, scalar2=0.0,
                                op0=ALU.add, op1=ALU.max)
                        bk[0] += 1
                        for oo in range(o0, o + 1):
                            Rinfo[(bb, oo, x)] = (re, 512 * (oo - o0))
                return fn

            def emit_C(bb, o):
                def fn():
                    if o % psc_w == 0:
                        state["ogc"] = ps_c.tile([128, 512 * psc_w], F32,
                                                 tag="c", name="ogc")
                    ogc = state["ogc"]
                    for j, x in enumerate(("A", "B")):
                        re, coff = Rinfo.pop((bb, o, x))
                        nc.tensor.matmul(ogc[:, ds(512 * (o % psc_w), 512)],
                                         s_sb[j], re[:, ds(coff, 512)],
                                         start=(j == 0), stop=(j == 1))
                    if o % psc_w != psc_w - 1:
                        return
                    og = opool.tile([128, 512 * psc_w], F32, name="og")
                    on_act_dr = (drk[0] % 8 < ad8) if ad8 is not None else (
                        drk[0] % 4 < dr_act)
                    if on_act_dr:
                        nc.scalar.activation(out=og, in_=ogc,
                                             func=AF.Identity, bias=dcon_ap)
                    else:
                        nc.vector.tensor_scalar(
                            out=og, in0=ogc, scalar1=dcon_ap, scalar2=None,
                            op0=ALU.add)
                    drk[0] += 1
                    Rinfo[(bb, o // psc_w, "og")] = og
                return fn

            def emit_out(bb, o2):
                def fn():
                    og = Rinfo.pop((bb, o2, "og"))
                    out_flat = out_d[bb].rearrange("h i j -> h (i j)")
                    for oo in range(psc_w):
                        dstv = out_flat[:, ds(4096 * (psc_w * o2 + oo), 4096)
                                        ].rearrange("h (t c) -> t h c", t=8)
                        src = og[:, ds(512 * oo, 512)]
                        if odk[0] % 4 < pool_dma:
                            nc.gpsimd.dma_start(out=dstv, in_=src)
                        else:
                            nc.sync.dma_start(out=dstv, in_=src)
                        odk[0] += 1
                return fn

            for bb in range(BPC):
                for ch in range(NO // uw):
                    at(bb, uw * ch - UA, 0, emit_u8(bb, ch))
                for o in range(NO):
                    at(bb, o - EA, 0, emit_bcast(bb, o))
                    at(bb, o - BA, 1, emit_basis(bb, o))
                    at(bb, o, 2, emit_C(bb, o))
                for o2 in range(NO // psc_w):
                    at(bb, psc_w * (o2 + 1) - 1 + out_delay, 3,
                       emit_out(bb, o2))

            for gstep in range(-PRE, BPC * NO):
                for _, fn in sorted(actions.pop(gstep, ()),
                                    key=lambda pf: pf[0]):
                    fn()
    nc.compile()
    return nc


# ---------------------------------------------------------------------------
# host-side fit for the trivial path
# ---------------------------------------------------------------------------
_FIT_CACHE = {}


def _softplus(x):
    return np.logaddexp(0.0, x)


def _make_true_phi(mu, log_sigma, W1, b1, W2, b2):
    sigma = _softplus(log_sigma) + 1e-6

    def true_phi(sv):
        x = (sv[:, None] - mu) / sigma
        psi = np.exp(-0.5 * x * x) / (np.sqrt(2.0 * np.pi) * sigma)
        hmid = np.maximum(psi @ W1 + b1, 0.0)
        return hmid @ W2 + b2

    return true_phi


def _fit_trivial(key, g0, b0, dmin, dmax, mu, log_sigma, W1, b1, W2, b2,
                 max_nfev=120):
    """Fit phi(g0*u + b0) ~= c0 + sum_m coef[m,:] relu(u - k_m) over
    u in [dmin, dmax].  Returns (knots[M], coef[M,H], c0[H])."""
    if key in _FIT_CACHE:
        return _FIT_CACHE[key]
    from scipy.optimize import least_squares

    true_phi = _make_true_phi(mu, log_sigma, W1, b1, W2, b2)
    G = 3000
    ug = np.linspace(dmin, dmax, G)
    y = true_phi(g0 * ug + b0)

    def Amat(kn, u):
        return np.concatenate([np.ones((len(u), 1)),
                               np.maximum(u[:, None] - kn, 0.0)], axis=1)

    def solve(kn):
        A = Amat(kn, ug)
        c, *_ = np.linalg.lstsq(A, y, rcond=None)
        return A, c

    kn0 = dmin + (dmax - dmin) * np.arange(M) / M

    def resid(kn):
        A, c = solve(np.sort(kn))
        return ((A @ c) - y).ravel()

    r = least_squares(resid, kn0, method="lm", max_nfev=max_nfev)
    kn = np.sort(r.x)
    _, c = solve(kn)
    _FIT_CACHE[key] = (kn, c[1:], c[0])
    return _FIT_CACHE[key]


def _pack_consts_trivial(knots, coef, c0):
    """Pack for the M=16-block kernel. Partition p = 16*t + m; basis split
    into sets A (knots[0:16]) and B (knots[16:32]); S_A/S_B block-diagonal:
    S_X[16t+m, 16t+h] = coefX[m, h]."""
    Cc = np.zeros((128, CW), dtype=np.float32)
    cfA = np.asarray(coef[:16], np.float32)              # [16, H]
    cfB = np.asarray(coef[16:], np.float32)
    for t in range(8):
        Cc[16 * t:16 * t + 16, 16 * t:16 * t + 16] = cfA
        Cc[16 * t:16 * t + 16, 128 + 16 * t:128 + 16 * t + 16] = cfB
    for b in range(8):                                   # sel8 rows
        Cc[b, 256 + 16 * b:256 + 16 * b + 16] = 1.0
    kn = np.asarray(knots, np.float32)
    Cc[:, 384] = -np.tile(kn[:16], 8)
    Cc[:, 385] = -np.tile(kn[16:], 8)
    Cc[:, 386] = np.tile(np.asarray(c0, np.float32), 8)
    return Cc


# ---------------------------------------------------------------------------
# general path: v1 kernel (kept verbatim from the previous version)
# ---------------------------------------------------------------------------
M1 = 128                   # v1 basis rows
CW1 = 1540                 # v1 packed const tile width
NSLAB = N * N // 512
NGRP = NSLAB // 8
ACT_SHARE = 69


def _build_nc_general():
    trivial = False
    nc = bacc.Bacc("TRN2", target_bir_lowering=False)

    d_in = nc.dram_tensor("d", [BPC, N, N], F32R if trivial else F32,
                          kind="ExternalInput")
    if not trivial:
        tokf = nc.dram_tensor("tokf", [BPC, N], F32R, kind="ExternalInput")
    c_d = nc.dram_tensor("consts", [128, CW1], F32R, kind="ExternalInput")
    out_d = nc.dram_tensor("out", [BPC, H, N, N], F32, kind="ExternalOutput")

    with tile.TileContext(nc) as tc:
        with (
            tc.tile_pool(name="consts", bufs=1) as cpool,
            tc.tile_pool(name="setup", bufs=2) as spool,
            tc.tile_pool(name="upool", bufs=2) as upool,
            tc.tile_pool(name="pairs", bufs=3) as ppool,
            tc.tile_pool(name="work", bufs=8) as wpool,
            tc.tile_pool(name="outp", bufs=4) as opool,
            tc.tile_pool(name="bcast", bufs=2) as bpool,
            tc.tile_pool(name="ps_e", bufs=5, space="PSUM") as ps_e,
            tc.tile_pool(name="ps_c", bufs=3, space="PSUM") as ps_c,
        ):
            C = cpool.tile([128, CW1], F32R)
            nc.sync.dma_start(out=C[:, 1280:CW1], in_=c_d[:, 1280:CW1])
            if not trivial:
                nc.sync.dma_start(out=C[:, 0:256], in_=c_d[:, 0:256])
                nc.sync.dma_start(out=C[:, 256:1280], in_=c_d[:, 256:1280])
            gT_sb = C[:, 0:128]
            bT_sb = C[:, 128:256]
            cC8_sb = C[:, 256:1280]
            slope_sb = C[0:1, 1280:1408]
            ones_sb = C[0:1, 1408:1536]
            iota_sb = C[:, 1536:1537].bitcast(F32)
            bias_sb = C[:, 1537:1538].bitcast(F32)

            wus = cpool.tile([1, 16], F32)
            nc.vector.tensor_scalar(
                out=wus[:, 0:8], in0=C[0:1, 1280:1288], scalar1=0.0,
                scalar2=None, op0=ALU.add,
            )
            nc.scalar.copy(out=wus[:, 8:16], in_=C[0:1, 1280:1288])
            wu = ps_e.tile([1, 8], F32, tag="e")
            nc.tensor.matmul(wu, C[0:1, 1280:1281].bitcast(F32),
                             C[0:1, 1280:1288].bitcast(F32),
                             start=True, stop=True)
            nc.vector.tensor_scalar(
                out=wus[:, 0:8], in0=wu, scalar1=0.0, scalar2=None,
                op0=ALU.add,
            )

            def setup1(bb):
                ctx = {"bb": bb, "pq": {}, "u": []}
                if trivial:
                    return ctx
                tok_sb = spool.tile([1, N], F32R, tag="tok")
                nc.sync.dma_start(out=tok_sb, in_=tokf[bb: bb + 1, :])
                tb_ps = ps_e.tile([T, N], F32, tag="e")
                nc.tensor.matmul(tb_ps, ones_sb, tok_sb, start=True, stop=True)
                ot_sb = spool.tile([T, N], F32R, tag="ot")
                nc.vector.tensor_scalar(
                    out=ot_sb, in0=tb_ps, scalar1=iota_sb, scalar2=None,
                    op0=ALU.is_equal,
                )
                ctx["ot"] = ot_sb
                return ctx

            def setup2(ctx):
                if trivial:
                    return
                ot_sb = ctx["ot"]
                ag_ps = ps_e.tile([T, N], F32, tag="e")
                nc.tensor.matmul(ag_ps, gT_sb, ot_sb, start=True, stop=True)
                ag_sb = spool.tile([T, N], F32R, tag="ag")
                nc.scalar.copy(out=ag_sb, in_=ag_ps)
                ab_ps = ps_e.tile([T, N], F32, tag="e")
                nc.tensor.matmul(ab_ps, bT_sb, ot_sb, start=True, stop=True)
                ab_sb = spool.tile([T, N], F32R, tag="ab")
                nc.scalar.copy(out=ab_sb, in_=ab_ps)
                ctx["ag"] = ag_sb
                ctx["ab"] = ab_sb

            def setup3(ctx, hh):
                if trivial:
                    return
                bb, ot_sb = ctx["bb"], ctx["ot"]
                rows = ds(128 * hh, 128)
                dh_sb = spool.tile([128, N], F32, tag="d")
                nc.sync.dma_start(
                    out=dh_sb, in_=d_in[bb, 128 * hh: 128 * hh + 128, :]
                )
                g_ps = ps_e.tile([128, N], F32, tag="e")
                nc.tensor.matmul(g_ps, ot_sb[:, rows], ctx["ag"],
                                 start=True, stop=True)
                bt_ps = ps_e.tile([128, N], F32, tag="e")
                nc.tensor.matmul(bt_ps, ot_sb[:, rows], ctx["ab"],
                                 start=True, stop=True)
                u_sb = upool.tile([128, N], F32R)
                nc.vector.tensor_tensor(
                    out=u_sb, in0=dh_sb, in1=g_ps, op=ALU.mult
                )
                nc.vector.tensor_tensor(
                    out=u_sb, in0=u_sb, in1=bt_ps, op=ALU.add
                )
                ctx["u"].append(u_sb)

            def stage(ctx, e):
                pt = ppool.tile([1, 32 * N], F32R, name="pq")
                if trivial:
                    nc.sync.dma_start(
                        out=pt, in_=d_in[ctx["bb"], 32 * e: 32 * e + 32, :]
                    )
                else:
                    hh, qq = divmod(e, 4)
                    nc.sync.dma_start(
                        out=pt, in_=ctx["u"][hh][ds(32 * qq, 32), :]
                    )
                ctx["pq"][e] = pt

            relu_k_box = [0]
            ctx = setup1(0)
            setup2(ctx)
            setup3(ctx, 0)
            setup3(ctx, 1)
            stage(ctx, 0)
            stage(ctx, 1)

            def make_qkind(counts):
                acc = {k: 0.0 for k in counts}
                total = sum(counts.values())
                out = []
                for q in range(total):
                    for k in counts:
                        acc[k] += counts[k] / total
                    pick = max(acc, key=lambda k: acc[k])
                    acc[pick] -= 1.0
                    out.append(pick)
                return out

            QKIND = make_qkind({"P": 25, "D": 16, "E": 23})
            d_flat = d_in.rearrange("b i j -> b (i j)") if trivial else None

            def quad_kind(bbi, g, half):
                if not trivial:
                    return "E"
                return QKIND[(bbi * 2 * NGRP + g * 2 + half) % 64]

            dq_ub = {}

            def emit_dbcast(bbi, g):
                for half in range(2):
                    if quad_kind(bbi, g, half) != "D":
                        continue
                    t0 = 4 * half
                    ub = bpool.tile([128, 2048], F32R, name="ubd",
                                    tag="ubd", bufs=4)
                    src = d_flat[
                        bbi: bbi + 1, ds(512 * (8 * g + t0), 2048)
                    ].broadcast_to([128, 2048])
                    nc.sync.dma_start(out=ub, in_=src)
                    dq_ub[(bbi, g, half)] = ub

            def emit_pbcast(fctx, g):
                bbi = fctx["bb"]
                pq = fctx["pq"][g // 2]
                for half in range(2):
                    if quad_kind(bbi, g, half) != "P":
                        continue
                    t0 = 4 * half
                    sl = (8 * g + t0) % 16
                    ub = bpool.tile([128, 2048], F32R, name="ub")
                    nc.gpsimd.partition_broadcast(
                        ub, pq[:, ds(512 * sl, 2048)]
                    )
                    dq_ub[(bbi, g, half)] = ub

            def emit_front(fctx, g):
                pq = fctx["pq"][g // 2]
                bbi = fctx["bb"]
                info = [None] * 8
                for half in range(2):
                    t0 = 4 * half
                    sl = (8 * g + t0) % 16
                    kq = quad_kind(bbi, g, half)
                    if kq in ("P", "D"):
                        ub = dq_ub.pop((bbi, g, half))
                        r2 = wpool.tile([128, 2048], F32R, tag="r2",
                                        bufs=5)
                        nc.vector.tensor_scalar(
                            out=r2, in0=ub, scalar1=bias_sb,
                            scalar2=0.0, op0=ALU.add, op1=ALU.max,
                        )
                        for j in range(4):
                            info[t0 + j] = (r2, 512 * j)
                        continue
                    for t in range(t0, t0 + 4):
                        sl = (8 * g + t) % 16
                        e_ps = ps_e.tile([M1, 512], F32, tag="e")
                        nc.tensor.matmul(
                            e_ps, slope_sb,
                            pq[:, ds(512 * sl, 512)],
                            start=True, stop=True,
                        )
                        r_sb = wpool.tile([M1, 512], F32R, bufs=4)
                        if (relu_k_box[0] * ACT_SHARE) % 92 < ACT_SHARE:
                            nc.scalar.activation(
                                out=r_sb, in_=e_ps, func=AF.Relu,
                                bias=bias_sb,
                            )
                        else:
                            nc.vector.tensor_scalar(
                                out=r_sb, in0=e_ps, scalar1=bias_sb,
                                scalar2=0.0, op0=ALU.add, op1=ALU.max,
                            )
                        relu_k_box[0] += 1
                        info[t] = (r_sb, 0)
                return info

            DBC_AHEAD = 3
            PBC_AHEAD = 2
            if trivial:
                nc.sync.dma_start(out=C[:, 256:1280], in_=c_d[:, 256:1280])
                for gg in range(DBC_AHEAD):
                    emit_dbcast(0, gg)
                for gg in range(PBC_AHEAD):
                    emit_pbcast(ctx, gg)
            front = emit_front(ctx, 0)

            for bb in range(BPC):
                out_flat = out_d[bb].rearrange("h i j -> h (i j)")
                nxt_ctx = None

                for g in range(NGRP):
                    nxt = g // 2 + 2
                    if g % 2 == 0 and nxt < 8:
                        stage(ctx, nxt)
                    if bb + 1 < BPC:
                        if g == 6:
                            nxt_ctx = setup1(bb + 1)
                        elif g == 8:
                            setup2(nxt_ctx)
                        elif g == 10:
                            setup3(nxt_ctx, 0)
                        elif g == 11:
                            setup3(nxt_ctx, 1)
                        elif g == 13:
                            stage(nxt_ctx, 0)
                        elif g == 14:
                            stage(nxt_ctx, 1)
                    if trivial:
                        ga = g + DBC_AHEAD
                        if ga < NGRP:
                            emit_dbcast(bb, ga)
                        elif bb + 1 < BPC:
                            emit_dbcast(bb + 1, ga - NGRP)
                        gp = g + PBC_AHEAD
                        if gp < NGRP:
                            emit_pbcast(ctx, gp)
                        elif bb + 1 < BPC and nxt_ctx is not None:
                            emit_pbcast(nxt_ctx, gp - NGRP)
                    if g + 1 < NGRP:
                        nfront = emit_front(ctx, g + 1)
                    elif bb + 1 < BPC:
                        nfront = emit_front(nxt_ctx, 0)
                    else:
                        nfront = None
                    ogc = ps_c.tile([128, 512], F32, tag="c")
                    for t in range(8):
                        r_tile, coff = front[t]
                        nc.tensor.matmul(
                            ogc, cC8_sb[:, ds(128 * t, 128)],
                            r_tile[:, ds(coff, 512)],
                            start=(t == 0), stop=(t == 7),
                        )
                    front = nfront
                    og = opool.tile([128, 512], F32)
                    nc.scalar.activation(out=og, in_=ogc, func=AF.Copy)
                    dst = out_flat[:, ds(4096 * g, 4096)].rearrange(
                        "h (t c) -> t h c", t=8
                    )
                    nc.sync.dma_start(out=dst, in_=og[:, :])
                ctx = nxt_ctx
    nc.compile()
    return nc


_NC_CACHE = {}


def _get_nc(trivial=True, cfg=None):
    key = (trivial, tuple(sorted((cfg or {}).items(), key=repr)))
    if key not in _NC_CACHE:
        _NC_CACHE[key] = (_build_nc_trivial(cfg) if trivial
                          else _build_nc_general())
    return _NC_CACHE[key]


def kernel(d, tokens, mu, log_sigma, W1, b1, W2, b2, gamma_table, beta_table,
           _cfg=None):
    d = np.ascontiguousarray(np.asarray(d), dtype=np.float32)
    d = np.nan_to_num(d, nan=0.0, posinf=0.0, neginf=0.0)
    tokens = np.asarray(tokens)
    mu = np.asarray(mu, dtype=np.float64)
    log_sigma = np.asarray(log_sigma, dtype=np.float64)
    W1 = np.asarray(W1, dtype=np.float64)
    b1 = np.asarray(b1, dtype=np.float64)
    W2 = np.asarray(W2, dtype=np.float64)
    b2 = np.asarray(b2, dtype=np.float64)
    gamma_table = np.asarray(gamma_table, dtype=np.float64)
    beta_table = np.asarray(beta_table, dtype=np.float64)

    g0 = float(gamma_table.flat[0])
    b0 = float(beta_table.flat[0])
    trivial = bool(np.all(gamma_table == g0) and np.all(beta_table == b0))

    dmin = float(d.min())
    dmax = float(d.max())

    if trivial:
        cfg = dict(_cfg or {})
        knots, coef, c0 = _fit_trivial(
            ("t", g0, b0, round(dmin, 9), round(dmax, 9)),
            g0, b0, dmin, dmax, mu, log_sigma, W1, b1, W2, b2)
        Cc = _pack_consts_trivial(knots, coef, c0)
        in_maps = []
        for c in range(NCORES):
            in_maps.append({
                "consts": Cc,
                "d": np.ascontiguousarray(d[BPC * c: BPC * (c + 1)]),
            })
        nc = _get_nc(True, cfg)
        res = run_bass_kernel_spmd(nc, in_maps, list(range(NCORES))).results
        out = np.concatenate([res[c]["out"] for c in range(NCORES)], axis=0)
        return out.astype(np.float32)

    # ---- general path (v1) ----
    cand = np.stack([gamma_table * dmin + beta_table,
                     gamma_table * dmax + beta_table])
    s_lo = float(cand.min())
    s_hi = float(cand.max())
    R = max(s_hi - s_lo, 1e-6)

    true_phi = _make_true_phi(mu, log_sigma, W1, b1, W2, b2)
    G = 8192
    sg = np.linspace(s_lo, s_hi, G)
    phig = true_phi(sg)
    ug = sg - s_lo
    knots = R * np.arange(1, M1 - 1) / (M1 - 1)
    A = np.concatenate(
        [np.ones((G, 1)), ug[:, None],
         np.maximum(ug[:, None] - knots, 0.0)], axis=1)
    coefC, *_ = np.linalg.lstsq(A, phig, rcond=None)
    slopes = np.concatenate([[0.0, 1.0], np.ones(M1 - 2)])
    biases = np.concatenate([[1.0, 0.0], -knots])

    Cc = np.zeros((128, CW1), dtype=np.float32)
    Cc[:, 0:128] = gamma_table.T.astype(np.float32)
    Cc[:, 128:256] = (beta_table - s_lo).T.astype(np.float32)
    cf = coefC.astype(np.float32)
    for t in range(8):
        base = 256 + 128 * t + 16 * t
        Cc[:, base: base + 16] = cf
    Cc[0, 1280:1408] = slopes.astype(np.float32)
    Cc[0, 1408:1536] = 1.0
    Cc[:, 1536] = np.arange(T, dtype=np.float32)
    Cc[:, 1537] = biases.astype(np.float32)

    common = {"consts": Cc}
    in_maps = []
    for c in range(NCORES):
        m = dict(common)
        m["d"] = np.ascontiguousarray(d[BPC * c: BPC * (c + 1)])
        m["tokf"] = np.ascontiguousarray(
            tokens.astype(np.float32)[BPC * c: BPC * (c + 1)])
        in_maps.append(m)

    nc = _get_nc(False)
    res = run_bass_kernel_spmd(nc, in_maps, list(range(NCORES))).results
    out = np.concatenate([res[c]["out"] for c in range(NCORES)], axis=0)
    return out.astype(np.float32)
